# revision 9
# baseline (speedup 1.0000x reference)
"""Trainium2 Bass kernel for nn_DQNConv (conv stack -> linear -> legal-move
masked softmax), data-parallel over 8 NeuronCores.

Self-contained: takes FULL inputs as numpy arrays, shards batch across the 8
cores, runs one SPMD Bass program, returns the FULL [16384, 4096] float32
output.

Algorithm per core (2048 rows):
  - The three VALID 3x3 convs are expressed as dense matmuls with features on
    the SBUF partition dim and batch on the free dim (7x7x1 -> 800 -> 576 ->
    64), fused bias+relu on the PSUM->SBUF evacuation.
  - logits tile [128 rows, 4096] = feat_tile.T @ Wl.T via TensorE (f32r).
  - e = exp(logits) on ScalarE straight out of PSUM (logits are in [-3, 3]).
  - legal-move mask is scattered per-row by GPSIMD local_scatter (writes 1.0
    at each index; duplicate indices write the same value, so last-write-wins
    is exact; chunks of <=1366 columns due to the op's num_elems limit).
  - u = e * mask with a fused row-sum Z (VectorE tensor_tensor_reduce), then
    out = u * (1/Z) and a straight HWDGE DMA to HBM.  Illegal entries are
    exactly 0.0, matching the reference (exp(-1000-ish) underflows to 0).
"""

import sys
import os

for _p in ("/opt/trn_rl_repo", "/root/.axon_site/_ro/trn_rl_repo"):
    if os.path.isdir(_p) and _p not in sys.path:
        sys.path.append(_p)

import numpy as np

import concourse.bass as bass
import concourse.bacc as bacc
import concourse.mybir as mybir
import concourse.tile as tile
from concourse.bass_utils import run_bass_kernel_spmd

B, HW, OUT, K = 16384, 7, 4096, 64
NCORES = 8
BC = B // NCORES           # 2048 rows per core
NRT = BC // 128            # 16 row-tiles per core
NCHUNK = 4                 # conv batch chunks per core
CW = BC // NCHUNK          # 512 columns per conv chunk
F0, F1, F2, F3 = 49, 800, 576, 64
SCATTER_CHUNKS = [(0, 2046), (2046, 2046), (4092, 4)]

dt = mybir.dt
AT = mybir.AluOpType
ACTF = mybir.ActivationFunctionType
F32R = dt.float32r

# dtype of the exp/mask/u tiles; bf16 halves DVE cost, fp32 is more precise
E_DT = dt.bfloat16 if os.environ.get("KERNEL_E_DT", "bf16") == "bf16" else dt.float32


def _ptiles(n):
    """Split a feature count into partition tiles of <=128."""
    out = []
    base = 0
    while base < n:
        out.append((base, min(128, n - base)))
        base += 128
    return out


def _build(reps=1):
    nc = bacc.Bacc("TRN2", target_bir_lowering=False, debug=False)

    xT = nc.dram_tensor("xT", [F0, BC], F32R, kind="ExternalInput")
    m1 = nc.dram_tensor("m1", [F0, F1], F32R, kind="ExternalInput")
    m2 = nc.dram_tensor("m2", [F1, F2], F32R, kind="ExternalInput")
    m3 = nc.dram_tensor("m3", [F2, F3], F32R, kind="ExternalInput")
    wlT = nc.dram_tensor("wlT", [F3 + 1, OUT], F32R, kind="ExternalInput")
    b1d = nc.dram_tensor("b1v", [F1, 1], dt.float32, kind="ExternalInput")
    b2d = nc.dram_tensor("b2v", [F2, 1], dt.float32, kind="ExternalInput")
    b3d = nc.dram_tensor("b3v", [F3, 1], dt.float32, kind="ExternalInput")
    pmi = nc.dram_tensor("pmi", [BC, 3 * K], dt.int16, kind="ExternalInput")
    outd = nc.dram_tensor("out", [BC, OUT], dt.float32, kind="ExternalOutput")

    t1 = _ptiles(F1)   # 7 tiles: 6x128 + 32
    t2 = _ptiles(F2)   # 5 tiles: 4x128 + 64
    KF = F3 + 1        # 65 = features + homogeneous 1-row for bl

    with tile.TileContext(nc) as tc:
        with (
            tc.tile_pool(name="w", bufs=1) as wp,
            tc.tile_pool(name="h", bufs=2) as hp,
            tc.tile_pool(name="b", bufs=2) as bp,
            tc.tile_pool(name="ps", bufs=2, space="PSUM") as pp,
        ):
            # ---- static loads -------------------------------------------------
            xT_sb = wp.tile([F0, BC], F32R, tag="xT")
            nc.sync.dma_start(out=xT_sb[:], in_=xT.ap())
            m1_sb = wp.tile([F0, F1], F32R, tag="m1")
            nc.sync.dma_start(out=m1_sb[:], in_=m1.ap())
            m2_sb = []
            for i, (kb, kn) in enumerate(t1):
                t = wp.tile([kn, F2], F32R, tag=f"m2_{i}")
                nc.sync.dma_start(out=t[:], in_=m2.ap()[kb:kb + kn, :])
                m2_sb.append(t)
            m3_sb = []
            for i, (kb, kn) in enumerate(t2):
                t = wp.tile([kn, F3], F32R, tag=f"m3_{i}")
                nc.sync.dma_start(out=t[:], in_=m3.ap()[kb:kb + kn, :])
                m3_sb.append(t)
            wl_sb = wp.tile([KF, OUT], F32R, tag="wl")
            nc.sync.dma_start(out=wl_sb[:], in_=wlT.ap())
            # biases: load as per-tile [pn, 1] tiles
            b_tiles = {}
            for name, drt, tl in (("b1", b1d, t1), ("b2", b2d, t2), ("b3", b3d, _ptiles(F3))):
                for i, (kb, kn) in enumerate(tl):
                    t = wp.tile([kn, 1], dt.float32, tag=f"{name}_{i}")
                    nc.sync.dma_start(out=t[:], in_=drt.ap()[kb:kb + kn, :])
                    b_tiles[(name, i)] = t
            # all 16 row-tiles of scatter indices in one load
            ix_sb = wp.tile([128, NRT, 3 * K], dt.int16, tag="ix")
            nc.sync.dma_start(
                out=ix_sb[:],
                in_=pmi.ap().rearrange("(t p) j -> p t j", p=128),
            )
            ones_sb = wp.tile([128, K], E_DT, tag="ones")
            nc.vector.memset(ones_sb[:], 1.0)

            # ---- per-chunk conv + per-row-tile softmax ------------------------
            for _rep in range(reps):
              for c in range(NCHUNK):
                cs = slice(c * CW, (c + 1) * CW)

                # L1: [49 x 800] -> h1 (relu(x@M1 + b1))
                h1 = []
                for i, (kb, kn) in enumerate(t1):
                    ps = pp.tile([kn, CW], dt.float32, tag="ps")
                    nc.tensor.matmul(
                        ps[:],
                        m1_sb[:, kb:kb + kn],
                        xT_sb[:, cs],
                        start=True, stop=True,
                    )
                    h = hp.tile([kn, CW], F32R, tag=f"h1_{i}")
                    nc.vector.tensor_scalar(
                        out=h[:], in0=ps[:], scalar1=b_tiles[("b1", i)][:],
                        scalar2=0.0, op0=AT.add, op1=AT.max)
                    h1.append(h)

                # L2: [800 x 576]
                h2 = []
                for i, (mb, mn) in enumerate(t2):
                    ps = pp.tile([mn, CW], dt.float32, tag="ps")
                    for kt, (kb, kn) in enumerate(t1):
                        nc.tensor.matmul(
                            ps[:],
                            m2_sb[kt][:, mb:mb + mn],
                            h1[kt][:],
                            start=(kt == 0), stop=(kt == len(t1) - 1),
                        )
                    h = hp.tile([mn, CW], F32R, tag=f"h2_{i}")
                    nc.vector.tensor_scalar(
                        out=h[:], in0=ps[:], scalar1=b_tiles[("b2", i)][:],
                        scalar2=0.0, op0=AT.add, op1=AT.max)
                    h2.append(h)

                # L3: [576 x 64] -> feat chunk [65, CW] (row 64 = ones)
                ps3 = pp.tile([F3, CW], dt.float32, tag="ps")
                for kt, (kb, kn) in enumerate(t2):
                    nc.tensor.matmul(
                        ps3[:],
                        m3_sb[kt][:],
                        h2[kt][:],
                        start=(kt == 0), stop=(kt == len(t2) - 1),
                    )
                feat = hp.tile([KF, CW], F32R, tag="feat")
                nc.vector.tensor_scalar(
                    out=feat[:F3, :], in0=ps3[:], scalar1=b_tiles[("b3", 0)][:],
                    scalar2=0.0, op0=AT.add, op1=AT.max)
                nc.vector.memset(feat[F3:KF, :].bitcast(dt.float32), 1.0)

                # ---- phase B: 4 row-tiles of this chunk -----------------------
                for r in range(CW // 128):
                    rt = c * (CW // 128) + r
                    lhsT = feat[:, r * 128:(r + 1) * 128]

                    e = bp.tile([128, OUT], E_DT, tag="e")
                    for half in range(2):
                        psl = pp.tile([128, OUT // 2], dt.float32, tag="ps")
                        for nb in range(4):
                            ns = slice(half * 2048 + nb * 512,
                                       half * 2048 + (nb + 1) * 512)
                            nc.tensor.matmul(
                                psl[:, nb * 512:(nb + 1) * 512],
                                lhsT,
                                wl_sb[:, ns],
                                start=True, stop=True,
                            )
                        nc.scalar.activation(
                            e[:, half * 2048:(half + 1) * 2048], psl[:], ACTF.Exp)

                    msk = bp.tile([128, OUT], E_DT, tag="msk")
                    for ci, (base, size) in enumerate(SCATTER_CHUNKS):
                        nc.gpsimd.local_scatter(
                            out_ap=msk[:, base:base + size],
                            data_ap=ones_sb[:],
                            idxs_ap=ix_sb[:, rt, ci * K:(ci + 1) * K],
                            channels=128, num_elems=size, num_idxs=K)

                    u = bp.tile([128, OUT], E_DT, tag="u")
                    z = bp.tile([128, 1], dt.float32, tag="z")
                    nc.vector.scalar_tensor_tensor(
                        out=u[:], in0=e[:], scalar=1.0, in1=msk[:],
                        op0=AT.mult, op1=AT.mult, accum_out=z[:])
                    rz = bp.tile([128, 1], dt.float32, tag="rz")
                    nc.vector.reciprocal(rz[:], z[:])
                    o = bp.tile([128, OUT], dt.float32, tag="o")
                    nc.vector.tensor_scalar(
                        out=o[:], in0=u[:], scalar1=rz[:], scalar2=None,
                        op0=AT.mult)
                    nc.sync.dma_start(
                        out=outd.ap()[rt * 128:(rt + 1) * 128, :], in_=o[:])

    nc.compile()
    return nc


_CACHE = {}


def _get_nc(reps=1):
    key = ("nc", reps)
    if key not in _CACHE:
        _CACHE[key] = _build(reps)
    return _CACHE[key]


def _conv_mats(W1, W2, W3):
    """Dense [in_feat, out_feat] matrices for the three VALID 3x3 convs with
    channel-major (c, y, x) feature flattening on both sides."""
    M1 = np.zeros((F0, F1), np.float32)
    for ky in range(3):
        for kx in range(3):
            for oy in range(5):
                for ox in range(5):
                    # row = input pixel, col = (oc, oy, ox)
                    M1[(oy + ky) * 7 + (ox + kx),
                       np.arange(32) * 25 + oy * 5 + ox] = W1[:, 0, ky, kx]
    M2 = np.zeros((F1, F2), np.float32)
    ic = np.arange(32)
    for ky in range(3):
        for kx in range(3):
            for oy in range(3):
                for ox in range(3):
                    rows = ic * 25 + (oy + ky) * 5 + (ox + kx)      # [32]
                    cols = np.arange(64) * 9 + oy * 3 + ox           # [64]
                    M2[np.ix_(rows, cols)] = W2[:, :, ky, kx].T      # [32,64]
    M3 = W3.transpose(1, 2, 3, 0).reshape(F2, F3).astype(np.float32)
    return M1, M2, M3


def kernel(**inputs):
    x = np.ascontiguousarray(np.asarray(inputs["x"], dtype=np.float32)).reshape(B, F0)
    pm = np.asarray(inputs["possible_moves"]).astype(np.int32, copy=False)
    W1 = np.asarray(inputs["W1"], dtype=np.float32)
    b1 = np.asarray(inputs["b1"], dtype=np.float32)
    W2 = np.asarray(inputs["W2"], dtype=np.float32)
    b2 = np.asarray(inputs["b2"], dtype=np.float32)
    W3 = np.asarray(inputs["W3"], dtype=np.float32)
    b3 = np.asarray(inputs["b3"], dtype=np.float32)
    Wl = np.asarray(inputs["Wl"], dtype=np.float32)
    bl = np.asarray(inputs["bl"], dtype=np.float32)

    M1, M2, M3 = _conv_mats(W1, W2, W3)
    WlT = np.concatenate([Wl.T.astype(np.float32), bl[None, :]], axis=0)
    b1v = np.repeat(b1, 25).reshape(F1, 1).astype(np.float32)
    b2v = np.repeat(b2, 9).reshape(F2, 1).astype(np.float32)
    b3v = b3.reshape(F3, 1).astype(np.float32)

    # per-row scatter indices, chunked to local_scatter's num_elems limit
    pmi = np.empty((B, 3, K), np.int16)
    for ci, (base, size) in enumerate(SCATTER_CHUNKS):
        inr = (pm >= base) & (pm < base + size)
        pmi[:, ci, :] = np.where(inr, pm - base, -1).astype(np.int16)
    pmi = pmi.reshape(B, 3 * K)

    xTall = np.ascontiguousarray(x.T)   # [49, B]

    nc = _get_nc()
    in_maps = []
    for c in range(NCORES):
        sl = slice(c * BC, (c + 1) * BC)
        in_maps.append({
            "xT": np.ascontiguousarray(xTall[:, sl]),
            "m1": M1, "m2": M2, "m3": M3, "wlT": WlT,
            "b1v": b1v, "b2v": b2v, "b3v": b3v,
            "pmi": np.ascontiguousarray(pmi[sl]),
        })

    trace = bool(int(os.environ.get("KERNEL_TRACE", "0")))
    res = run_bass_kernel_spmd(nc, in_maps, list(range(NCORES)), trace=trace)
    _CACHE["last_results"] = res
    out = np.concatenate([res.results[i]["out"] for i in range(NCORES)], axis=0)
    return out
